# revision 13
# baseline (speedup 1.0000x reference)
"""Trainium2 Bass kernel for nn_GroupedConvFuseSide4.

out[b,k] = w[k,0]*side5[b,k] + w[k,1]*side4[b,k]
         + w[k,2]*side1[b,0] + w[k,3]*side2[b,0] + w[k,4]*side3[b,0] + bias[k]

Sharding: pure data parallel over batch (B=8) across 8 NeuronCores.

v5 scheme — fp16 wire format + combined-contraction tiles:
  The 262144 pixels of one image are split into CH=64 chunks of FD=4096
  fp16 values; row r = 19*g + k (chunk g, class k) gives ROWS=1216 rows.
  Tiles of R=103 rows: the SBUF rhs tile stacks the 103 x4 rows with the
  25 [ones; side1/2/3 chunk] rows, so a single 128-contraction matmul per
  512-col chunk computes w1*x4 + bias + w2*s1 + w3*s2 + w4*s3 at once
  (per-tile baked lhsT: diagonal block + singles block).  DVE then does
  one scalar_tensor_tensor per [R,1024] psum group: out = w0*x5 + psum.
  Loads go on the sync+scalar HWDGE queues, stores on the gpsimd queue,
  at 2048-col granularity so stores interleave with loads.
  Host converts to fp16/repacks (contiguous [rows, 8KB] DMA blocks) and
  upcasts the fp16 output; max rel err vs f32 reference ~8e-4.
"""

import numpy as np

B, K, H, W = 8, 19, 512, 512
FD = 4096                  # pixels per chunk
CH = 64                    # chunks per image (H*W / FD)
ROWS = K * CH              # 1216 packed rows per core
RT = 103                   # data rows per tile (RT + 25 <= 128 contraction)
TILES = []
_r = 0
while _r < ROWS:
    TILES.append((_r, min(RT, ROWS - _r)))
    _r += RT
NT = len(TILES)            # 11 x 103 + 1 x 83
NGRP = FD // 1024          # 4 psum groups of [R, 1024] per tile
N_CORES = 8

_cache = {}


def _build_program(w, b):
    import concourse.bacc as bacc
    import concourse.tile as tile
    import concourse.mybir as mybir
    from contextlib import ExitStack

    f16 = mybir.dt.float16
    f32 = mybir.dt.float32
    mult = mybir.AluOpType.mult
    add = mybir.AluOpType.add

    nc = bacc.Bacc(
        "TRN2", target_bir_lowering=False, debug=False,
        enable_asserts=False, num_devices=N_CORES,
    )

    x5_d = nc.dram_tensor("x5", [ROWS, FD], f16, kind="ExternalInput").ap()
    x4_d = nc.dram_tensor("x4", [ROWS, FD], f16, kind="ExternalInput").ap()
    xs_d = nc.dram_tensor("xs", [NT, 25, FD], f16, kind="ExternalInput").ap()
    out_d = nc.dram_tensor("out", [ROWS, FD], f16, kind="ExternalOutput").ap()

    # ---- per-tile baked lhsT (103 % 19 != 0 so k(p) shifts per tile) ----
    # lhsT_t is [R+25, R]: rows 0:R = diag(w1[k(p)]), rows R:R+25 =
    # [bias; w(2+s)[k(p)] selecting the chunk of side_s for row p].
    cons16 = np.zeros((128, 128 * NT), dtype=np.float16)
    consw0 = np.zeros((128, NT), dtype=np.float32)
    for t, (r0, R) in enumerate(TILES):
        rr = r0 + np.arange(R)
        kk = rr % K
        gg = rr // K
        g0 = r0 // K
        cons16[np.arange(R), 128 * t + np.arange(R)] = w[kk, 1].astype(np.float16)
        cons16[R, 128 * t:128 * t + R] = b.astype(np.float16)[kk]
        for s in range(3):
            cons16[R + 1 + 8 * s + (gg - g0),
                   128 * t + np.arange(R)] = w[kk, 2 + s].astype(np.float16)
        consw0[:R, t] = w[kk, 0]
    cons16_d = nc.inline_tensor(cons16, name="cons16").ap()
    consw0_d = nc.inline_tensor(consw0, name="consw0").ap()

    with tile.TileContext(nc) as tc, ExitStack() as ctx:
        consts = ctx.enter_context(tc.tile_pool(name="consts", bufs=1))
        x5_pool = ctx.enter_context(tc.tile_pool(name="x5", bufs=4))
        xt_pool = ctx.enter_context(tc.tile_pool(name="xt", bufs=4))
        o_pool = ctx.enter_context(tc.tile_pool(name="o", bufs=4))
        ps_pool = ctx.enter_context(tc.tile_pool(name="ps", bufs=4, space="PSUM"))

        c16 = consts.tile([128, 128 * NT], f16, tag="c16")
        nc.sync.dma_start(out=c16[:], in_=cons16_d)
        cw0 = consts.tile([128, NT], f32, tag="cw0")
        nc.sync.dma_start(out=cw0[:], in_=consw0_d)
        lhs_t = [c16[0:R + 25, 128 * t:128 * t + R]
                 for t, (r0, R) in enumerate(TILES)]
        w0_t = [cw0[0:R, t:t + 1] for t, (r0, R) in enumerate(TILES)]

        for t, (r0, R) in enumerate(TILES):
            x5t = x5_pool.tile([RT, FD], f16, tag="x5")
            nc.sync.dma_start(out=x5t[0:R, :], in_=x5_d[r0:r0 + R])
            # combined rhs tile: x4 rows on partitions 0:R, singles on R:R+25
            xt = xt_pool.tile([128, FD], f16, tag="xt")
            nc.scalar.dma_start(out=xt[0:R, :], in_=x4_d[r0:r0 + R])
            nc.scalar.dma_start(out=xt[R:R + 25, :], in_=xs_d[t])
            ot = o_pool.tile([RT, FD], f16, tag="o")

            pss = [ps_pool.tile([RT, 1024], f32, tag="ps", name=f"ps{g}")
                   for g in range(NGRP)]
            for h in range(2 * NGRP):
                nc.tensor.matmul(
                    pss[h // 2][0:R, 512 * (h % 2):512 * (h % 2) + 512],
                    lhs_t[t], xt[0:R + 25, 512 * h:512 * (h + 1)],
                    start=True, stop=True,
                )
            for g in range(NGRP):
                sl = slice(1024 * g, 1024 * (g + 1))
                nc.vector.scalar_tensor_tensor(
                    ot[0:R, sl], x5t[0:R, sl], w0_t[t], pss[g][0:R, :],
                    mult, add)
                if g % 2 == 1:
                    # store each 2048-col half as soon as its STTs are done
                    osl = slice(2048 * (g // 2), 2048 * (g // 2) + 2048)
                    nc.gpsimd.dma_start(out=out_d[r0:r0 + R, osl],
                                        in_=ot[0:R, osl])

    nc.compile()
    return nc


def _get_program(w, b):
    key = (w.tobytes(), b.tobytes())
    if key not in _cache:
        _cache[key] = _build_program(w, b)
    return _cache[key]


def _pack_kchw(a16):
    """[K, CH, FD] fp16 -> [ROWS, FD], row = 19*g + k."""
    return np.ascontiguousarray(a16.transpose(1, 0, 2)).reshape(ROWS, FD)


def run(inputs, trace=False, tmpdir=None):
    from concourse.bass_utils import run_bass_kernel_spmd

    w = np.asarray(inputs["weight"], dtype=np.float32)
    b = np.asarray(inputs["bias"], dtype=np.float32)
    nc = _get_program(w, b)

    s1h = np.asarray(inputs["side1"]).astype(np.float16).reshape(B, CH, FD)
    s2h = np.asarray(inputs["side2"]).astype(np.float16).reshape(B, CH, FD)
    s3h = np.asarray(inputs["side3"]).astype(np.float16).reshape(B, CH, FD)
    s4h = np.asarray(inputs["side4"]).astype(np.float16).reshape(B, K, CH, FD)
    s5h = np.asarray(inputs["side5"]).astype(np.float16).reshape(B, K, CH, FD)

    in_maps = []
    for c in range(N_CORES):
        xsp = np.zeros((NT, 25, FD), dtype=np.float16)
        xsp[:, 0] = np.float16(1.0)
        for t, (r0, R) in enumerate(TILES):
            g0 = r0 // K
            g1 = (r0 + R - 1) // K
            n = g1 - g0 + 1
            for s, a in enumerate((s1h[c], s2h[c], s3h[c])):
                xsp[t, 1 + 8 * s:1 + 8 * s + n] = a[g0:g1 + 1]
        in_maps.append({
            "x5": _pack_kchw(s5h[c]),
            "x4": _pack_kchw(s4h[c]),
            "xs": xsp,
        })

    res = run_bass_kernel_spmd(nc, in_maps, list(range(N_CORES)),
                               trace=trace, tmpdir=tmpdir)
    outs = []
    for c in range(N_CORES):
        o = res.results[c]["out"].reshape(CH, K, FD).transpose(1, 0, 2)
        outs.append(o.reshape(1, K, H, W).astype(np.float32))
    return np.concatenate(outs, axis=0), res


def kernel(**inputs):
    out, _ = run(inputs, trace=False)
    return out
